# revision 1
# baseline (speedup 1.0000x reference)
"""Trainium2 Bass kernel for MinimalLinearAttention.

  q = relu(x @ q_w.T + q_b); k = relu(x @ k_w.T + k_b); v = x @ v_w.T + v_b
  kv[b,h] = sum_s k[b,s,h,:] outer v[b,s,h,:]          (per batch, all tokens)
  out[b,s,h] = q[b,s,h,:] @ kv[b,h]
  y = out @ o_w.T + o_b

Sharding: token-parallel over 8 cores. Each core takes a 512-token slice of
every batch (2048 tokens), computes k/v projections + partial kv, AllReduces
kv across cores (per batch, overlapped with the q projection), then does the
q readout + output projection for its own tokens. Host concatenates slices.

Matmuls run as float32r (TF32-like single-pass, 1 cyc/row at N>=256) with
fp32 PSUM accumulation. Walrus requires float32r matmul operands to be
produced by a rounding instruction, so PE-feeding tiles are allocated as
float32r: PSUM evictions (ACT/DVE) round for free; DMA-loaded tensors get
one DVE conversion copy.

On-device layouts (per core):
  xt   = x_slice.T            [D=1024, T=2048]   (T cols batch-major: b*512+s)
  wq/wk/wv/wo = W.T           [Din=1024, Dout=1024]
  K, V                        [T, D]     (from xt-stationary matmuls)
  Q^T                         [D, T]     (from w-stationary matmuls)
  kv per (batch, head-pair)   [128, 128] block-diagonal (2 heads of 64)
  O^T                         [D, T]
  y                           [T, D]
"""

import os
import sys

os.environ.setdefault("MYCRO_LOCAL_CACHE", "1")

for _p in ("/opt/trn_rl_repo", "/root/.axon_site/_ro/trn_rl_repo"):
    if os.path.isdir(_p) and _p not in sys.path:
        sys.path.insert(0, _p)

import numpy as np

B, S, D, H, HD = 4, 4096, 1024, 16, 64
NCORES = 8
SC = S // NCORES          # 512 tokens per core per batch
T = B * SC                # 2048 rows per core
NPAIR = 8                 # head pairs (2 heads of 64 dims = 128 partitions)
NDIN = D // 128           # 8 Din tiles
NT = T // 128             # 16 T tiles per core
NTB = SC // 128           # 4 T tiles per batch

_CACHE = {}


def build_program():
    """f32r variant (v2): fast-start DMA order, wq prefetch, per-batch
    fused output projection, diagonal-packed kv collectives."""
    if "nc_f32r" in _CACHE:
        return _CACHE["nc_f32r"]

    import concourse.bacc as bacc
    import concourse.tile as tile
    from concourse import bass, mybir

    f32 = mybir.dt.float32
    FR = mybir.dt.float32r
    RELU = mybir.ActivationFunctionType.Relu

    nc = bacc.Bacc("TRN2", target_bir_lowering=False, debug=False,
                   num_devices=NCORES)

    xt_d = nc.dram_tensor("xt", [D, T], f32, kind="ExternalInput").ap()
    wq_d = nc.dram_tensor("wq", [D, D], f32, kind="ExternalInput").ap()
    wk_d = nc.dram_tensor("wk", [D, D], f32, kind="ExternalInput").ap()
    wv_d = nc.dram_tensor("wv", [D, D], f32, kind="ExternalInput").ap()
    wo_d = nc.dram_tensor("wo", [D, D], f32, kind="ExternalInput").ap()
    bq_d = nc.dram_tensor("bq", [128, NDIN], f32, kind="ExternalInput").ap()
    bk_d = nc.dram_tensor("bk", [1, D], f32, kind="ExternalInput").ap()
    bv_d = nc.dram_tensor("bv", [1, D], f32, kind="ExternalInput").ap()
    bo_d = nc.dram_tensor("bo", [1, D], f32, kind="ExternalInput").ap()
    y_d = nc.dram_tensor("y", [T, D], f32, kind="ExternalOutput").ap()

    HPB = 16 * 64  # bounce rows per batch: 16 heads x 64 d-rows

    from contextlib import ExitStack

    with tile.TileContext(nc) as tc:
        with ExitStack() as top:
            constp = top.enter_context(tc.tile_pool(name="const", bufs=1))
            rawp = top.enter_context(tc.tile_pool(name="raw", bufs=3))
            dramp = top.enter_context(
                tc.tile_pool(name="dram", bufs=1, space="DRAM"))
            psp = top.enter_context(
                tc.tile_pool(name="ps", bufs=3, space="PSUM"))
            pskvp = top.enter_context(
                tc.tile_pool(name="pskv", bufs=4, space="PSUM"))

            def load_fr(pool, dram_ap, shape, tag, name):
                raw = rawp.tile(shape, f32, tag="raw", name=f"raw_{name}")
                nc.sync.dma_start(raw[:], dram_ap)
                t = pool.tile(shape, FR, tag=tag, name=name)
                nc.vector.tensor_copy(t[:], raw[:])
                return t

            ones_raw = constp.tile([1, 128], f32, tag="ones_raw")
            nc.vector.memset(ones_raw[:], 1.0)
            ones = constp.tile([1, 128], FR, tag="ones")
            nc.vector.tensor_copy(ones[:], ones_raw[:])
            bq_sb = constp.tile([128, NDIN], f32, tag="bq")
            nc.sync.dma_start(bq_sb[:], bq_d[:])

            bnc_in = [dramp.tile([HPB, 64], f32, tag=f"bi{b}",
                                 name=f"bnc_in{b}") for b in range(B)]
            bnc_out = [dramp.tile([HPB, 64], f32, tag=f"bo{b}",
                                  addr_space="Shared", name=f"bnc_out{b}")
                       for b in range(B)]

            with tc.tile_pool(name="xtp", bufs=1) as xtp:
                # xt first chunks (cols 0:1024) + wk: minimal set for the
                # first matmuls; then wv, then xt second chunks
                xts = []
                for dn in range(NDIN):
                    t = xtp.tile([128, T], FR, tag=f"xt{dn}",
                                 name=f"xt_sb{dn}")
                    raw = rawp.tile([128, 1024], f32, tag="raw",
                                    name=f"raw_xt{dn}_0")
                    nc.sync.dma_start(raw[:], xt_d[dn * 128:(dn + 1) * 128,
                                                   0:1024])
                    nc.vector.tensor_copy(t[:, 0:1024], raw[:])
                    xts.append(t)

                with tc.tile_pool(name="wqpre", bufs=1) as wqprep, \
                        ExitStack() as st1:
                    wkvp = st1.enter_context(tc.tile_pool(name="wkv", bufs=1))
                    kvbp = st1.enter_context(tc.tile_pool(name="kvb", bufs=3))
                    kvexp = st1.enter_context(tc.tile_pool(name="kvex", bufs=8))
                    wk_sb = [load_fr(wkvp, wk_d[dn * 128:(dn + 1) * 128, :],
                                     [128, D], f"wk{dn}", f"wk_sb{dn}")
                             for dn in range(NDIN)]
                    bk_sb = load_fr(wkvp, bk_d[:], [1, D], "bk", "bk_sb")
                    wv_sb = [load_fr(wkvp, wv_d[dn * 128:(dn + 1) * 128, :],
                                     [128, D], f"wv{dn}", f"wv_sb{dn}")
                             for dn in range(NDIN)]
                    bv_sb = load_fr(wkvp, bv_d[:], [1, D], "bv", "bv_sb")
                    for dn in range(NDIN):
                        raw = rawp.tile([128, 1024], f32, tag="raw",
                                        name=f"raw_xt{dn}_1")
                        nc.sync.dma_start(
                            raw[:], xt_d[dn * 128:(dn + 1) * 128, 1024:2048])
                        nc.vector.tensor_copy(xts[dn][:, 1024:2048], raw[:])

                    wq_sb = [None] * NDIN

                    for b in range(B):
                        kvps = [pskvp.tile([128, 512], f32, tag="kvps",
                                           name=f"kvps{b}_{w}")
                                for w in range(4)]
                        for t in range(NTB):
                            gt = b * NTB + t
                            kt = kvbp.tile([128, D], FR, tag="kb")
                            vt = kvbp.tile([128, D], FR, tag="vb")
                            for w_sb, brow, dst, act in (
                                (wk_sb, bk_sb, kt, "relu"),
                                (wv_sb, bv_sb, vt, "copy"),
                            ):
                                for hf in range(2):
                                    ps = psp.tile([128, 512], f32, tag="ps")
                                    for dn in range(NDIN):
                                        nc.tensor.matmul(
                                            ps[:],
                                            xts[dn][:, gt * 128:(gt + 1) * 128],
                                            w_sb[dn][:, hf * 512:(hf + 1) * 512],
                                            start=(dn == 0), stop=False)
                                    nc.tensor.matmul(
                                        ps[:], ones[:, 0:128],
                                        brow[:, hf * 512:(hf + 1) * 512],
                                        start=False, stop=True)
                                    dsl = dst[:, hf * 512:(hf + 1) * 512]
                                    if act == "relu":
                                        nc.scalar.activation(dsl, ps[:], RELU)
                                    else:
                                        nc.vector.tensor_copy(dsl, ps[:])
                            for p in range(NPAIR):
                                nc.tensor.matmul(
                                    kvps[p // 2][:, (p % 2) * 256:(p % 2) * 256 + 256],
                                    kt[:, p * 128:(p + 1) * 128],
                                    vt[:, (p // 2) * 256:(p // 2) * 256 + 256],
                                    start=(t == 0 and p % 2 == 0),
                                    stop=(t == NTB - 1 and p % 2 == 1))
                        # ship only diagonal [64,64] blocks (head h = 2p+j)
                        for p in range(NPAIR):
                            for j in range(2):
                                ex = kvexp.tile([64, 64], f32, tag="kvex",
                                                name=f"kvex{b}_{p}_{j}")
                                nc.vector.tensor_copy(
                                    ex[:],
                                    kvps[p // 2][j * 64:(j + 1) * 64,
                                                 (p % 2) * 384 + j * 64:
                                                 (p % 2) * 384 + j * 64 + 64])
                                h = 2 * p + j
                                nc.sync.dma_start(
                                    bnc_in[b][h * 64:(h + 1) * 64, :], ex[:])
                        nc.gpsimd.collective_compute(
                            "AllReduce", mybir.AluOpType.add,
                            replica_groups=[list(range(NCORES))],
                            ins=[bnc_in[b].opt()], outs=[bnc_out[b].opt()])
                        if b == 1:
                            # prefetch the first wq tiles into spare SBUF so
                            # stage 2 starts without a weight-load bubble
                            for dn in range(2):
                                wq_sb[dn] = load_fr(
                                    wqprep, wq_d[dn * 128:(dn + 1) * 128, :],
                                    [128, D], f"wqp{dn}", f"wq_sb{dn}")

                    # ---- Stage 2: Q^T proj + readout + fused o-proj ----
                    st1.close()
                    with ExitStack() as st2:
                        wq2p = st2.enter_context(
                            tc.tile_pool(name="wq2", bufs=1))
                        wop = st2.enter_context(tc.tile_pool(name="wo", bufs=1))
                        otbp = st2.enter_context(
                            tc.tile_pool(name="otb", bufs=1))
                        qtp = st2.enter_context(tc.tile_pool(name="qt", bufs=3))
                        kvrawp = st2.enter_context(
                            tc.tile_pool(name="kvraw", bufs=4))
                        kvsbp = st2.enter_context(
                            tc.tile_pool(name="kvsb", bufs=8))
                        ytp = st2.enter_context(tc.tile_pool(name="yt", bufs=3))
                        for dn in range(2, NDIN):
                            wq_sb[dn] = load_fr(
                                wq2p, wq_d[dn * 128:(dn + 1) * 128, :],
                                [128, D], f"wq{dn}", f"wq_sb{dn}")
                        wo_sb = [load_fr(wop, wo_d[dn * 128:(dn + 1) * 128, :],
                                         [128, D], f"wo{dn}", f"wo_sb{dn}")
                                 for dn in range(NDIN)]
                        bo_sb = load_fr(wop, bo_d[:], [1, D], "bo", "bo_sb")

                        for b in range(B):
                            otb = [otbp.tile([128, 512], FR, tag=f"otb{p}",
                                             name=f"otb{b}_{p}")
                                   for p in range(NPAIR)]
                            for p in range(NPAIR):
                                raw = kvrawp.tile([128, 128], f32, tag="kvraw",
                                                  name=f"kvraw{p}_{b}")
                                nc.vector.memset(raw[:], 0.0)
                                for j in range(2):
                                    h = 2 * p + j
                                    nc.sync.dma_start(
                                        raw[j * 64:(j + 1) * 64,
                                            j * 64:(j + 1) * 64],
                                        bnc_out[b][h * 64:(h + 1) * 64, :])
                                kvsb = kvsbp.tile([128, 128], FR, tag="kvsb",
                                                  name=f"kvsb{p}_{b}")
                                nc.vector.tensor_copy(kvsb[:], raw[:])
                                ps = psp.tile([128, 512], f32, tag="ps")
                                for dn in range(NDIN):
                                    nc.tensor.matmul(
                                        ps[:],
                                        wq_sb[dn][:, p * 128:(p + 1) * 128],
                                        xts[dn][:, b * 512:(b + 1) * 512],
                                        start=(dn == 0), stop=(dn == NDIN - 1))
                                qt = qtp.tile([128, 512], FR, tag="qt")
                                nc.scalar.activation(qt[:], ps[:], RELU,
                                                     bias=bq_sb[:, p:p + 1])
                                pso = psp.tile([128, 512], f32, tag="ps")
                                nc.tensor.matmul(pso[:], kvsb[:], qt[:],
                                                 start=True, stop=True)
                                nc.vector.tensor_copy(otb[p][:], pso[:])
                            # fused output projection for this batch
                            for t in range(NTB):
                                gt = b * NTB + t
                                yt = ytp.tile([128, D], f32, tag="yt")
                                for hf in range(2):
                                    ps = psp.tile([128, 512], f32, tag="ps")
                                    for dn in range(NDIN):
                                        nc.tensor.matmul(
                                            ps[:],
                                            otb[dn][:, t * 128:(t + 1) * 128],
                                            wo_sb[dn][:, hf * 512:(hf + 1) * 512],
                                            start=(dn == 0), stop=False)
                                    nc.tensor.matmul(
                                        ps[:], ones[:, 0:128],
                                        bo_sb[:, hf * 512:(hf + 1) * 512],
                                        start=False, stop=True)
                                    nc.vector.tensor_copy(
                                        yt[:, hf * 512:(hf + 1) * 512], ps[:])
                                nc.sync.dma_start(
                                    y_d[gt * 128:(gt + 1) * 128, :], yt[:])

    nc.compile()
    _CACHE["nc_f32r"] = nc
    return nc


def build_program_bf16():
    """bf16 variant: all matmul operands bf16 (host-cast), flat SBUF layout
    with every weight resident, DMA ordering for fast PE start, and
    diagonal-packed kv collectives."""
    if "nc_bf16" in _CACHE:
        return _CACHE["nc_bf16"]

    import concourse.bacc as bacc
    import concourse.tile as tile
    from concourse import bass, mybir

    f32 = mybir.dt.float32
    BF = mybir.dt.bfloat16
    RELU = mybir.ActivationFunctionType.Relu

    nc = bacc.Bacc("TRN2", target_bir_lowering=False, debug=False,
                   num_devices=NCORES)

    xt_d = nc.dram_tensor("xt", [D, T], BF, kind="ExternalInput").ap()
    wq_d = nc.dram_tensor("wq", [D, D], BF, kind="ExternalInput").ap()
    wk_d = nc.dram_tensor("wk", [D, D], BF, kind="ExternalInput").ap()
    wv_d = nc.dram_tensor("wv", [D, D], BF, kind="ExternalInput").ap()
    wo_d = nc.dram_tensor("wo", [D, D], BF, kind="ExternalInput").ap()
    bq_d = nc.dram_tensor("bq", [128, NDIN], f32, kind="ExternalInput").ap()
    bk_d = nc.dram_tensor("bk", [1, D], BF, kind="ExternalInput").ap()
    bv_d = nc.dram_tensor("bv", [1, D], BF, kind="ExternalInput").ap()
    bo_d = nc.dram_tensor("bo", [1, D], BF, kind="ExternalInput").ap()
    y_d = nc.dram_tensor("y", [T, D], f32, kind="ExternalOutput").ap()

    HPB = 16 * 64  # bounce rows per batch: 16 heads x 64 d-rows

    with tile.TileContext(nc) as tc:
        with (
            tc.tile_pool(name="const", bufs=1) as constp,
            tc.tile_pool(name="wp", bufs=1) as wp,
            tc.tile_pool(name="xtp", bufs=1) as xtp,
            tc.tile_pool(name="otp", bufs=1) as otp,
            tc.tile_pool(name="kvb", bufs=3) as kvbp,
            tc.tile_pool(name="qt", bufs=4) as qtp,
            tc.tile_pool(name="kvex", bufs=8) as kvexp,
            tc.tile_pool(name="kvraw", bufs=4) as kvrawp,
            tc.tile_pool(name="kvsb", bufs=8) as kvsbp,
            tc.tile_pool(name="yt", bufs=3) as ytp,
            tc.tile_pool(name="dram", bufs=1, space="DRAM") as dramp,
            tc.tile_pool(name="ps", bufs=3, space="PSUM") as psp,
            tc.tile_pool(name="pskv", bufs=4, space="PSUM") as pskvp,
        ):
            # load order = scheduling priority: xt + wk first so the PE can
            # start, then wv, then wq/wo for the later stages
            xts = []
            for dn in range(NDIN):
                t = xtp.tile([128, T], BF, tag=f"xt{dn}", name=f"xt_sb{dn}")
                nc.sync.dma_start(t[:], xt_d[dn * 128:(dn + 1) * 128, :])
                xts.append(t)

            def loadw(dram_ap, tag):
                w = []
                for dn in range(NDIN):
                    t = wp.tile([128, D], BF, tag=f"{tag}{dn}",
                                name=f"{tag}_sb{dn}")
                    nc.sync.dma_start(t[:], dram_ap[dn * 128:(dn + 1) * 128, :])
                    w.append(t)
                return w

            wk_sb = loadw(wk_d, "wk")
            ones = constp.tile([1, 128], BF, tag="ones")
            nc.vector.memset(ones[:], 1.0)
            bk_sb = constp.tile([1, D], BF, tag="bk")
            nc.sync.dma_start(bk_sb[:], bk_d[:])
            wv_sb = loadw(wv_d, "wv")
            bv_sb = constp.tile([1, D], BF, tag="bv")
            nc.sync.dma_start(bv_sb[:], bv_d[:])
            wq_sb = loadw(wq_d, "wq")
            bq_sb = constp.tile([128, NDIN], f32, tag="bq")
            nc.sync.dma_start(bq_sb[:], bq_d[:])
            wo_sb = loadw(wo_d, "wo")
            bo_sb = constp.tile([1, D], BF, tag="bo")
            nc.sync.dma_start(bo_sb[:], bo_d[:])

            bnc_in = [dramp.tile([HPB, 64], f32, tag=f"bi{b}",
                                 name=f"bnc_in{b}") for b in range(B)]
            bnc_out = [dramp.tile([HPB, 64], f32, tag=f"bo{b}",
                                  addr_space="Shared", name=f"bnc_out{b}")
                       for b in range(B)]

            # ---- Stage 1: K,V projections + per-batch partial kv ----
            for b in range(B):
                kvps = [pskvp.tile([128, 512], f32, tag="kvps",
                                   name=f"kvps{b}_{w}") for w in range(4)]
                for t in range(NTB):
                    gt = b * NTB + t
                    kt = kvbp.tile([128, D], BF, tag="kb")
                    vt = kvbp.tile([128, D], BF, tag="vb")
                    for w_sb, brow, dst, act in (
                        (wk_sb, bk_sb, kt, "relu"),
                        (wv_sb, bv_sb, vt, "copy"),
                    ):
                        for hf in range(2):
                            ps = psp.tile([128, 512], f32, tag="ps")
                            for dn in range(NDIN):
                                nc.tensor.matmul(
                                    ps[:],
                                    xts[dn][:, gt * 128:(gt + 1) * 128],
                                    w_sb[dn][:, hf * 512:(hf + 1) * 512],
                                    start=(dn == 0), stop=False)
                            nc.tensor.matmul(
                                ps[:], ones[:, 0:128],
                                brow[:, hf * 512:(hf + 1) * 512],
                                start=False, stop=True)
                            dsl = dst[:, hf * 512:(hf + 1) * 512]
                            if act == "relu":
                                nc.scalar.activation(dsl, ps[:], RELU)
                            else:
                                nc.vector.tensor_copy(dsl, ps[:])
                    for p in range(NPAIR):
                        nc.tensor.matmul(
                            kvps[p // 2][:, (p % 2) * 256:(p % 2) * 256 + 256],
                            kt[:, p * 128:(p + 1) * 128],
                            vt[:, (p // 2) * 256:(p // 2) * 256 + 256],
                            start=(t == 0 and p % 2 == 0),
                            stop=(t == NTB - 1 and p % 2 == 1))
                # ship only the diagonal [64,64] blocks (head h = 2p+j)
                for p in range(NPAIR):
                    for j in range(2):
                        ex = kvexp.tile([64, 64], f32, tag="kvex",
                                        name=f"kvex{b}_{p}_{j}")
                        nc.vector.tensor_copy(
                            ex[:],
                            kvps[p // 2][j * 64:(j + 1) * 64,
                                         (p % 2) * 384 + j * 64:
                                         (p % 2) * 384 + j * 64 + 64])
                        h = 2 * p + j
                        nc.sync.dma_start(
                            bnc_in[b][h * 64:(h + 1) * 64, :], ex[:])
                nc.gpsimd.collective_compute(
                    "AllReduce", mybir.AluOpType.add,
                    replica_groups=[list(range(NCORES))],
                    ins=[bnc_in[b].opt()], outs=[bnc_out[b].opt()])

            # ---- Stage 2: Q^T projection + kv readout -> O^T ----
            ot_tiles = []
            for p in range(NPAIR):
                ot = otp.tile([128, T], BF, tag=f"ot{p}", name=f"ot{p}")
                ot_tiles.append(ot)
                for b in range(B):
                    raw = kvrawp.tile([128, 128], f32, tag="kvraw",
                                      name=f"kvraw{p}_{b}")
                    nc.vector.memset(raw[:], 0.0)
                    for j in range(2):
                        h = 2 * p + j
                        nc.sync.dma_start(
                            raw[j * 64:(j + 1) * 64, j * 64:(j + 1) * 64],
                            bnc_out[b][h * 64:(h + 1) * 64, :])
                    kvsb = kvsbp.tile([128, 128], BF, tag="kvsb",
                                      name=f"kvsb{p}_{b}")
                    nc.vector.tensor_copy(kvsb[:], raw[:])
                    ps = psp.tile([128, 512], f32, tag="ps")
                    for dn in range(NDIN):
                        nc.tensor.matmul(
                            ps[:],
                            wq_sb[dn][:, p * 128:(p + 1) * 128],
                            xts[dn][:, b * 512:(b + 1) * 512],
                            start=(dn == 0), stop=(dn == NDIN - 1))
                    qt = qtp.tile([128, 512], BF, tag="qt")
                    nc.scalar.activation(qt[:], ps[:], RELU,
                                         bias=bq_sb[:, p:p + 1])
                    pso = psp.tile([128, 512], f32, tag="ps")
                    nc.tensor.matmul(pso[:], kvsb[:], qt[:],
                                     start=True, stop=True)
                    nc.vector.tensor_copy(
                        ot[:, b * 512:(b + 1) * 512], pso[:])

            # ---- Stage 3: output projection y = O @ o_w.T + o_b ----
            for gt in range(NT):
                yt = ytp.tile([128, D], f32, tag="yt")
                for hf in range(2):
                    ps = psp.tile([128, 512], f32, tag="ps")
                    for dn in range(NDIN):
                        nc.tensor.matmul(
                            ps[:],
                            ot_tiles[dn][:, gt * 128:(gt + 1) * 128],
                            wo_sb[dn][:, hf * 512:(hf + 1) * 512],
                            start=(dn == 0), stop=False)
                    nc.tensor.matmul(
                        ps[:], ones[:, 0:128],
                        bo_sb[:, hf * 512:(hf + 1) * 512],
                        start=False, stop=True)
                    nc.vector.tensor_copy(yt[:, hf * 512:(hf + 1) * 512], ps[:])
                nc.sync.dma_start(y_d[gt * 128:(gt + 1) * 128, :], yt[:])

    nc.compile()
    _CACHE["nc_bf16"] = nc
    return nc


def prepare_in_maps(x, q_w, q_b, k_w, k_b, v_w, v_b, o_w, o_b, dtype="bf16"):
    if dtype == "bf16":
        import ml_dtypes
        mmdt = ml_dtypes.bfloat16
    else:
        mmdt = np.float32
    shared = {
        "wq": np.ascontiguousarray(q_w.T).astype(mmdt),
        "wk": np.ascontiguousarray(k_w.T).astype(mmdt),
        "wv": np.ascontiguousarray(v_w.T).astype(mmdt),
        "wo": np.ascontiguousarray(o_w.T).astype(mmdt),
        "bq": np.ascontiguousarray(q_b.reshape(NDIN, 128).T),
        "bk": k_b.reshape(1, D).astype(mmdt),
        "bv": v_b.reshape(1, D).astype(mmdt),
        "bo": o_b.reshape(1, D).astype(mmdt),
    }
    in_maps = []
    for c in range(NCORES):
        xs = x[:, c * SC:(c + 1) * SC, :].reshape(T, D)
        m = dict(shared)
        m["xt"] = np.ascontiguousarray(xs.T).astype(mmdt)
        in_maps.append(m)
    return in_maps


def gather_output(results):
    y = np.empty((B, S, D), dtype=np.float32)
    for c in range(NCORES):
        y[:, c * SC:(c + 1) * SC, :] = results[c]["y"].reshape(B, SC, D)
    return y


DTYPE = "f32r"


def run(inputs, trace=False, dtype=None, **kw):
    from concourse import bass_utils
    dtype = dtype or DTYPE
    nc = build_program_bf16() if dtype == "bf16" else build_program()
    in_maps = prepare_in_maps(**inputs, dtype=dtype)
    res = bass_utils.run_bass_kernel_spmd(
        nc, in_maps, core_ids=list(range(NCORES)), trace=trace, **kw)
    return gather_output(res.results), res


def kernel(**inputs):
    y, _ = run(inputs)
    return y



# revision 2
# speedup vs baseline: 1.2117x; 1.2117x over previous
"""Trainium2 Bass kernel for MinimalLinearAttention.

  q = relu(x @ q_w.T + q_b); k = relu(x @ k_w.T + k_b); v = x @ v_w.T + v_b
  kv[b,h] = sum_s k[b,s,h,:] outer v[b,s,h,:]          (per batch, all tokens)
  out[b,s,h] = q[b,s,h,:] @ kv[b,h]
  y = out @ o_w.T + o_b

Sharding: token-parallel over 8 cores. Each core takes a 512-token slice of
every batch (2048 tokens), computes k/v projections + partial kv, AllReduces
kv across cores (per batch), then does the q readout + output projection for
its own tokens. Host concatenates slices.

v3 (all-bf16): every matmul operand bf16 (host-cast), so LDWEIGHTS ~105ns
hides under the ~263ns/512-col matmul issue period (vs f32r's 218ns loads).
All four weight matrices stay resident in SBUF (12MB total with x), which
removes the stage-transition weight-load bubble. DMA order front-loads the
first K-projection's operands. V and O biases ride the PSUM eviction as DVE
tensor_add against host-replicated bias tiles (no ones-row matmuls); K keeps
the ones-row bias matmul (relu needs bias pre-activation along the free dim);
Q bias is a per-partition scalar on the activation eviction.

Stage 2 is ordered to hide the last kv AllReduce (~25us end-to-end, and the
bounce-buffer reads wait on ALL collectives' semaphore): two batches of
Q-projections (~34us of PE work independent of kv) run before the first
readout. Readouts for a batch run only after all its Q tiles evicted, so the
PE never stalls on the qt activation latency.

On-device layouts (per core):
  xt   = x_slice.T            [D=1024, T=2048]   (T cols batch-major: b*512+s)
  wq/wk/wv/wo = W.T           [Din=1024, Dout=1024]
  K, V                        [T, D]     (from xt-stationary matmuls)
  Q^T                         [D, T]     (from w-stationary matmuls)
  kv per (batch, head-pair)   [128, 128] block-diagonal (2 heads of 64)
  y                           [T, D] f32
"""

import os
import sys

os.environ.setdefault("MYCRO_LOCAL_CACHE", "1")

for _p in ("/opt/trn_rl_repo", "/root/.axon_site/_ro/trn_rl_repo"):
    if os.path.isdir(_p) and _p not in sys.path:
        sys.path.insert(0, _p)

import numpy as np

B, S, D, H, HD = 4, 4096, 1024, 16, 64
NCORES = 8
SC = S // NCORES          # 512 tokens per core per batch
T = B * SC                # 2048 rows per core
NPAIR = 8                 # head pairs (2 heads of 64 dims = 128 partitions)
NDIN = D // 128           # 8 Din tiles
NT = T // 128             # 16 T tiles per core
NTB = SC // 128           # 4 T tiles per batch

CC_BF16 = True            # bf16 kv collective payload

_CACHE = {}


def build_program_v3():
    if "nc_v3" in _CACHE:
        return _CACHE["nc_v3"]

    import concourse.bacc as bacc
    import concourse.tile as tile
    from concourse import bass, mybir

    f32 = mybir.dt.float32
    BF = mybir.dt.bfloat16
    CCDT = BF if CC_BF16 else f32
    RELU = mybir.ActivationFunctionType.Relu
    ADD = mybir.AluOpType.add

    nc = bacc.Bacc("TRN2", target_bir_lowering=False, debug=False,
                   num_devices=NCORES)

    xt_d = nc.dram_tensor("xt", [D, T], BF, kind="ExternalInput").ap()
    wq_d = nc.dram_tensor("wq", [D, D], BF, kind="ExternalInput").ap()
    wk_d = nc.dram_tensor("wk", [D, D], BF, kind="ExternalInput").ap()
    wv_d = nc.dram_tensor("wv", [D, D], BF, kind="ExternalInput").ap()
    wo_d = nc.dram_tensor("wo", [D, D], BF, kind="ExternalInput").ap()
    bq_d = nc.dram_tensor("bq", [128, NDIN], f32, kind="ExternalInput").ap()
    bk_d = nc.dram_tensor("bk", [1, D], BF, kind="ExternalInput").ap()
    bvr_d = nc.dram_tensor("bvr", [128, D], BF, kind="ExternalInput").ap()
    bor_d = nc.dram_tensor("bor", [128, D], BF, kind="ExternalInput").ap()
    y_d = nc.dram_tensor("y", [T, D], f32, kind="ExternalOutput").ap()

    HPB = 16 * 64  # bounce rows per batch: 16 heads x 64 d-rows

    with tile.TileContext(nc) as tc:
        with (
            tc.tile_pool(name="const", bufs=1) as constp,
            tc.tile_pool(name="wp", bufs=1) as wp,
            tc.tile_pool(name="xtp", bufs=1) as xtp,
            tc.tile_pool(name="kvb", bufs=4) as kvbp,
            tc.tile_pool(name="qt", bufs=16) as qtp,
            tc.tile_pool(name="otb", bufs=10) as otbp,
            tc.tile_pool(name="kvex", bufs=8) as kvexp,
            tc.tile_pool(name="kvsb", bufs=16) as kvsbp,
            tc.tile_pool(name="yt", bufs=3) as ytp,
            tc.tile_pool(name="dram", bufs=1, space="DRAM") as dramp,
            tc.tile_pool(name="ps", bufs=3, space="PSUM") as psp,
            tc.tile_pool(name="pskv", bufs=4, space="PSUM") as pskvp,
        ):
            # ---- loads, ordered so the first K-projection starts ASAP ----
            wk_sb = [wp.tile([128, D], BF, tag=f"wk{dn}", name=f"wk_sb{dn}")
                     for dn in range(NDIN)]
            for dn in range(NDIN):
                nc.sync.dma_start(wk_sb[dn][:, 0:512],
                                  wk_d[dn * 128:(dn + 1) * 128, 0:512])
            xts = [xtp.tile([128, T], BF, tag=f"xt{dn}", name=f"xt_sb{dn}")
                   for dn in range(NDIN)]
            for dn in range(NDIN):
                nc.sync.dma_start(xts[dn][:, 0:512],
                                  xt_d[dn * 128:(dn + 1) * 128, 0:512])
            ones = constp.tile([1, 128], BF, tag="ones")
            nc.vector.memset(ones[:], 1.0)
            bk_sb = constp.tile([1, D], BF, tag="bk")
            nc.sync.dma_start(bk_sb[:], bk_d[:])
            for dn in range(NDIN):
                nc.sync.dma_start(wk_sb[dn][:, 512:1024],
                                  wk_d[dn * 128:(dn + 1) * 128, 512:1024])

            def loadw(dram_ap, tag):
                w = []
                for dn in range(NDIN):
                    t = wp.tile([128, D], BF, tag=f"{tag}{dn}",
                                name=f"{tag}_sb{dn}")
                    nc.sync.dma_start(t[:], dram_ap[dn * 128:(dn + 1) * 128, :])
                    w.append(t)
                return w

            wv_sb = loadw(wv_d, "wv")
            bvr_sb = constp.tile([128, D], BF, tag="bvr")
            nc.sync.dma_start(bvr_sb[:], bvr_d[:])
            for dn in range(NDIN):
                nc.sync.dma_start(xts[dn][:, 512:1024],
                                  xt_d[dn * 128:(dn + 1) * 128, 512:1024])
            wq_sb = loadw(wq_d, "wq")
            bq_sb = constp.tile([128, NDIN], f32, tag="bq")
            nc.sync.dma_start(bq_sb[:], bq_d[:])
            for dn in range(NDIN):
                nc.sync.dma_start(xts[dn][:, 1024:2048],
                                  xt_d[dn * 128:(dn + 1) * 128, 1024:2048])
            wo_sb = loadw(wo_d, "wo")
            bor_sb = constp.tile([128, D], BF, tag="bor")
            nc.sync.dma_start(bor_sb[:], bor_d[:])

            bnc_in = [dramp.tile([HPB, 64], CCDT, tag=f"bi{b}",
                                 name=f"bnc_in{b}") for b in range(B)]
            bnc_out = [dramp.tile([HPB, 64], CCDT, tag=f"bo{b}",
                                  addr_space="Shared", name=f"bnc_out{b}")
                       for b in range(B)]

            # ---- Stage 1: K,V projections + per-batch partial kv ----
            for b in range(B):
                kvps = [pskvp.tile([128, 512], f32, tag="kvps",
                                   name=f"kvps{b}_{w}") for w in range(4)]
                for t in range(NTB):
                    gt = b * NTB + t
                    kt = kvbp.tile([128, D], BF, tag="kb")
                    vt = kvbp.tile([128, D], BF, tag="vb")
                    for hf in range(2):
                        ps = psp.tile([128, 512], f32, tag="ps")
                        for dn in range(NDIN):
                            nc.tensor.matmul(
                                ps[:],
                                xts[dn][:, gt * 128:(gt + 1) * 128],
                                wk_sb[dn][:, hf * 512:(hf + 1) * 512],
                                start=(dn == 0), stop=False)
                        nc.tensor.matmul(
                            ps[:], ones[:, 0:128],
                            bk_sb[:, hf * 512:(hf + 1) * 512],
                            start=False, stop=True)
                        nc.scalar.activation(
                            kt[:, hf * 512:(hf + 1) * 512], ps[:], RELU)
                    for hf in range(2):
                        ps = psp.tile([128, 512], f32, tag="ps")
                        for dn in range(NDIN):
                            nc.tensor.matmul(
                                ps[:],
                                xts[dn][:, gt * 128:(gt + 1) * 128],
                                wv_sb[dn][:, hf * 512:(hf + 1) * 512],
                                start=(dn == 0), stop=(dn == NDIN - 1))
                        nc.vector.tensor_tensor(
                            vt[:, hf * 512:(hf + 1) * 512], ps[:],
                            bvr_sb[:, hf * 512:(hf + 1) * 512], ADD)
                    for p in range(NPAIR):
                        nc.tensor.matmul(
                            kvps[p // 2][:, (p % 2) * 256:(p % 2) * 256 + 256],
                            kt[:, p * 128:(p + 1) * 128],
                            vt[:, (p // 2) * 256:(p // 2) * 256 + 256],
                            start=(t == 0 and p % 2 == 0),
                            stop=(t == NTB - 1 and p % 2 == 1))
                # ship only the diagonal [64,64] blocks (head h = 2p+j)
                for p in range(NPAIR):
                    for j in range(2):
                        ex = kvexp.tile([64, 64], CCDT, tag="kvex",
                                        name=f"kvex{b}_{p}_{j}")
                        nc.vector.tensor_copy(
                            ex[:],
                            kvps[p // 2][j * 64:(j + 1) * 64,
                                         (p % 2) * 384 + j * 64:
                                         (p % 2) * 384 + j * 64 + 64])
                        h = 2 * p + j
                        nc.sync.dma_start(
                            bnc_in[b][h * 64:(h + 1) * 64, :], ex[:])
                nc.gpsimd.collective_compute(
                    "AllReduce", mybir.AluOpType.add,
                    replica_groups=[list(range(NCORES))],
                    ins=[bnc_in[b].opt()], outs=[bnc_out[b].opt()])

            # ---- Stage 2/3: Q^T proj, kv readout, fused o-proj ----
            kvsb = {}
            qts = {}

            def prefetch(b):
                for p in range(NPAIR):
                    kv = kvsbp.tile([128, 128], BF, tag="kvsb",
                                    name=f"kvsb{b}_{p}")
                    nc.vector.memset(kv[:], 0.0)
                    for j in range(2):
                        h = 2 * p + j
                        nc.sync.dma_start(
                            kv[j * 64:(j + 1) * 64, j * 64:(j + 1) * 64],
                            bnc_out[b][h * 64:(h + 1) * 64, :])
                    kvsb[(b, p)] = kv

            def qproj(b):
                for p in range(NPAIR):
                    ps = psp.tile([128, 512], f32, tag="ps")
                    for dn in range(NDIN):
                        nc.tensor.matmul(
                            ps[:],
                            wq_sb[dn][:, p * 128:(p + 1) * 128],
                            xts[dn][:, b * 512:(b + 1) * 512],
                            start=(dn == 0), stop=(dn == NDIN - 1))
                    qt = qtp.tile([128, 512], BF, tag="qt",
                                  name=f"qt{b}_{p}")
                    nc.scalar.activation(qt[:], ps[:], RELU,
                                         bias=bq_sb[:, p:p + 1])
                    qts[(b, p)] = qt

            def ro_oproj(b):
                otb = [otbp.tile([128, 512], BF, tag="otb",
                                 name=f"otb{b}_{p}") for p in range(NPAIR)]
                for p in range(NPAIR):
                    pso = psp.tile([128, 512], f32, tag="ps")
                    nc.tensor.matmul(pso[:], kvsb[(b, p)][:], qts[(b, p)][:],
                                     start=True, stop=True)
                    nc.vector.tensor_copy(otb[p][:], pso[:])
                for t in range(NTB):
                    gt = b * NTB + t
                    yt = ytp.tile([128, D], f32, tag="yt")
                    for hf in range(2):
                        ps = psp.tile([128, 512], f32, tag="ps")
                        for dn in range(NDIN):
                            nc.tensor.matmul(
                                ps[:],
                                otb[dn][:, t * 128:(t + 1) * 128],
                                wo_sb[dn][:, hf * 512:(hf + 1) * 512],
                                start=(dn == 0), stop=(dn == NDIN - 1))
                        nc.vector.tensor_tensor(
                            yt[:, hf * 512:(hf + 1) * 512], ps[:],
                            bor_sb[:, hf * 512:(hf + 1) * 512], ADD)
                    nc.sync.dma_start(y_d[gt * 128:(gt + 1) * 128, :], yt[:])

            prefetch(0)
            prefetch(1)
            qproj(0)
            qproj(1)
            ro_oproj(0)
            prefetch(2)
            qproj(2)
            ro_oproj(1)
            prefetch(3)
            qproj(3)
            ro_oproj(2)
            ro_oproj(3)

    nc.compile()
    _CACHE["nc_v3"] = nc
    return nc


def prepare_in_maps_v3(x, q_w, q_b, k_w, k_b, v_w, v_b, o_w, o_b):
    import ml_dtypes
    BF = ml_dtypes.bfloat16
    shared = {
        "wq": np.ascontiguousarray(q_w.T).astype(BF),
        "wk": np.ascontiguousarray(k_w.T).astype(BF),
        "wv": np.ascontiguousarray(v_w.T).astype(BF),
        "wo": np.ascontiguousarray(o_w.T).astype(BF),
        "bq": np.ascontiguousarray(q_b.reshape(NDIN, 128).T.astype(np.float32)),
        "bk": k_b.reshape(1, D).astype(BF),
        "bvr": np.broadcast_to(v_b.reshape(1, D), (128, D)).astype(BF),
        "bor": np.broadcast_to(o_b.reshape(1, D), (128, D)).astype(BF),
    }
    in_maps = []
    for c in range(NCORES):
        xs = x[:, c * SC:(c + 1) * SC, :].reshape(T, D)
        m = dict(shared)
        m["xt"] = np.ascontiguousarray(xs.T).astype(BF)
        in_maps.append(m)
    return in_maps


def gather_output(results):
    y = np.empty((B, S, D), dtype=np.float32)
    for c in range(NCORES):
        y[:, c * SC:(c + 1) * SC, :] = results[c]["y"].reshape(B, SC, D)
    return y


DTYPE = "v3"


def build_for(dtype):
    return build_program_v3()


def prepare_for(inputs, dtype):
    return prepare_in_maps_v3(**inputs)


def run(inputs, trace=False, dtype=None, **kw):
    from concourse import bass_utils
    dtype = dtype or DTYPE
    nc = build_for(dtype)
    in_maps = prepare_for(inputs, dtype)
    res = bass_utils.run_bass_kernel_spmd(
        nc, in_maps, core_ids=list(range(NCORES)), trace=trace, **kw)
    return gather_output(res.results), res


def kernel(**inputs):
    y, _ = run(inputs)
    return y


# revision 4
# speedup vs baseline: 1.2332x; 1.0177x over previous
"""Trainium2 Bass kernel for MinimalLinearAttention.

  q = relu(x @ q_w.T + q_b); k = relu(x @ k_w.T + k_b); v = x @ v_w.T + v_b
  kv[b,h] = sum_s k[b,s,h,:] outer v[b,s,h,:]          (per batch, all tokens)
  out[b,s,h] = q[b,s,h,:] @ kv[b,h]
  y = out @ o_w.T + o_b

Sharding: token-parallel over 8 cores. Each core takes a 512-token slice of
every batch (2048 tokens), computes k/v projections + partial kv, AllReduces
kv across cores (per batch), then does the q readout + output projection for
its own tokens. Host concatenates slices.

v4 (all-bf16): every matmul operand bf16 (host-cast) so LDWEIGHTS (~105ns)
hides under the ~263ns/512-col matmul issue period. All weights resident in
SBUF. Startup DMAs are spread across the sync/scalar/vector/gpsimd queues
(descriptor issue is ~0.6us per DMA per queue, so one queue serializes the
start) and ordered by first use. No bias matmuls at all: K and V biases ride
the PSUM eviction as DVE tensor_tensor adds against host-replicated bias
tiles (K's relu then happens on the scalar engine from the SBUF temp), O bias
likewise, Q bias is a per-partition activation scalar. y is stored bf16 in
[128,512] halves right after each eviction to shorten the output tail.

Stage 2 is ordered to hide the last kv AllReduce (~25us end-to-end, and the
bounce-buffer reads wait on ALL collectives): two batches of Q-projections
(~34us of kv-independent PE work) run before the first readout; readouts for
a batch run only after all its Q tiles evicted so the PE never stalls on the
qt activation latency.

On-device layouts (per core):
  xt   = x_slice.T            [D=1024, T=2048]   (T cols batch-major: b*512+s)
  wq/wk/wv/wo = W.T           [Din=1024, Dout=1024]
  K, V                        [T, D]     (from xt-stationary matmuls)
  Q^T                         [D, T]     (from w-stationary matmuls)
  kv per (batch, head-pair)   [128, 128] block-diagonal (2 heads of 64)
  y                           [T, D] bf16 (host widens to f32)
"""

import os
import sys

os.environ.setdefault("MYCRO_LOCAL_CACHE", "1")

for _p in ("/opt/trn_rl_repo", "/root/.axon_site/_ro/trn_rl_repo"):
    if os.path.isdir(_p) and _p not in sys.path:
        sys.path.insert(0, _p)

import numpy as np

B, S, D, H, HD = 4, 4096, 1024, 16, 64
NCORES = 8
SC = S // NCORES          # 512 tokens per core per batch
T = B * SC                # 2048 rows per core
NPAIR = 8                 # head pairs (2 heads of 64 dims = 128 partitions)
NDIN = D // 128           # 8 Din tiles
NT = T // 128             # 16 T tiles per core
NTB = SC // 128           # 4 T tiles per batch

CC_BF16 = True            # bf16 kv collective payload

_CACHE = {}


def build_program_v4():
    if "nc_v4" in _CACHE:
        return _CACHE["nc_v4"]

    import concourse.bacc as bacc
    import concourse.tile as tile
    from concourse import bass, mybir

    f32 = mybir.dt.float32
    BF = mybir.dt.bfloat16
    CCDT = BF if CC_BF16 else f32
    RELU = mybir.ActivationFunctionType.Relu
    ADD = mybir.AluOpType.add

    nc = bacc.Bacc("TRN2", target_bir_lowering=False, debug=False,
                   num_devices=NCORES)

    xt_d = nc.dram_tensor("xt", [D, T], BF, kind="ExternalInput").ap()
    wq_d = nc.dram_tensor("wq", [D, D], BF, kind="ExternalInput").ap()
    wk_d = nc.dram_tensor("wk", [D, D], BF, kind="ExternalInput").ap()
    wv_d = nc.dram_tensor("wv", [D, D], BF, kind="ExternalInput").ap()
    wo_d = nc.dram_tensor("wo", [D, D], BF, kind="ExternalInput").ap()
    bq_d = nc.dram_tensor("bq", [128, NDIN], f32, kind="ExternalInput").ap()
    bkr_d = nc.dram_tensor("bkr", [128, D], BF, kind="ExternalInput").ap()
    bvr_d = nc.dram_tensor("bvr", [128, D], BF, kind="ExternalInput").ap()
    bor_d = nc.dram_tensor("bor", [128, D], BF, kind="ExternalInput").ap()
    y_d = nc.dram_tensor("y", [T, D], BF, kind="ExternalOutput").ap()

    HPB = 16 * 64  # bounce rows per batch: 16 heads x 64 d-rows

    with tile.TileContext(nc) as tc:
        with (
            tc.tile_pool(name="const", bufs=1) as constp,
            tc.tile_pool(name="wp", bufs=1) as wp,
            tc.tile_pool(name="xtp", bufs=1) as xtp,
            tc.tile_pool(name="kvb", bufs=4) as kvbp,
            tc.tile_pool(name="ktmp", bufs=3) as ktmpp,
            tc.tile_pool(name="qt", bufs=16) as qtp,
            tc.tile_pool(name="otb", bufs=10) as otbp,
            tc.tile_pool(name="kvex", bufs=8) as kvexp,
            tc.tile_pool(name="kvsb", bufs=16) as kvsbp,
            tc.tile_pool(name="yt", bufs=4) as ytp,
            tc.tile_pool(name="dram", bufs=1, space="DRAM") as dramp,
            tc.tile_pool(name="ps", bufs=4, space="PSUM") as psp,
            tc.tile_pool(name="pskv", bufs=4, space="PSUM") as pskvp,
        ):
            # ---- loads: ordered by first use, issue spread over 4 queues ----
            qs = [nc.sync, nc.scalar, nc.gpsimd]
            qi = [0]

            def dma(dst, src):
                qs[qi[0] % 3].dma_start(dst, src)
                qi[0] += 1

            bkr_sb = constp.tile([128, D], BF, tag="bkr")
            dma(bkr_sb[:], bkr_d[:])
            wk_sb = [wp.tile([128, D], BF, tag=f"wk{dn}", name=f"wk_sb{dn}")
                     for dn in range(NDIN)]
            for dn in range(NDIN):
                dma(wk_sb[dn][:], wk_d[dn * 128:(dn + 1) * 128, :])
            xts = [xtp.tile([128, T], BF, tag=f"xt{dn}", name=f"xt_sb{dn}")
                   for dn in range(NDIN)]
            for dn in range(NDIN):
                dma(xts[dn][:, 0:512], xt_d[dn * 128:(dn + 1) * 128, 0:512])

            def loadw(dram_ap, tag):
                w = []
                for dn in range(NDIN):
                    t = wp.tile([128, D], BF, tag=f"{tag}{dn}",
                                name=f"{tag}_sb{dn}")
                    dma(t[:], dram_ap[dn * 128:(dn + 1) * 128, :])
                    w.append(t)
                return w

            wv_sb = loadw(wv_d, "wv")
            bvr_sb = constp.tile([128, D], BF, tag="bvr")
            dma(bvr_sb[:], bvr_d[:])
            for dn in range(NDIN):
                dma(xts[dn][:, 512:1024], xt_d[dn * 128:(dn + 1) * 128,
                                               512:1024])
            wq_sb = loadw(wq_d, "wq")
            bq_sb = constp.tile([128, NDIN], f32, tag="bq")
            dma(bq_sb[:], bq_d[:])
            for dn in range(NDIN):
                dma(xts[dn][:, 1024:2048], xt_d[dn * 128:(dn + 1) * 128,
                                                1024:2048])
            wo_sb = loadw(wo_d, "wo")
            bor_sb = constp.tile([128, D], BF, tag="bor")
            dma(bor_sb[:], bor_d[:])

            bnc_in = [dramp.tile([HPB, 64], CCDT, tag=f"bi{b}",
                                 name=f"bnc_in{b}") for b in range(B)]
            bnc_out = [dramp.tile([HPB, 64], CCDT, tag=f"bo{b}",
                                  addr_space="Shared", name=f"bnc_out{b}")
                       for b in range(B)]

            # ---- Stage 1: K,V projections + per-batch partial kv ----
            for b in range(B):
                kvps = [pskvp.tile([128, 512], f32, tag="kvps",
                                   name=f"kvps{b}_{w}") for w in range(4)]
                for t in range(NTB):
                    gt = b * NTB + t
                    kt = kvbp.tile([128, D], BF, tag="kb")
                    vt = kvbp.tile([128, D], BF, tag="vb")
                    for hf in range(2):
                        ps = psp.tile([128, 512], f32, tag="ps")
                        for dn in range(NDIN):
                            nc.tensor.matmul(
                                ps[:],
                                xts[dn][:, gt * 128:(gt + 1) * 128],
                                wk_sb[dn][:, hf * 512:(hf + 1) * 512],
                                start=(dn == 0), stop=(dn == NDIN - 1))
                        ktmp = ktmpp.tile([128, 512], BF, tag="ktmp")
                        nc.vector.tensor_tensor(
                            ktmp[:], ps[:],
                            bkr_sb[:, hf * 512:(hf + 1) * 512], ADD)
                        nc.scalar.activation(
                            kt[:, hf * 512:(hf + 1) * 512], ktmp[:], RELU)
                    for hf in range(2):
                        ps = psp.tile([128, 512], f32, tag="ps")
                        for dn in range(NDIN):
                            nc.tensor.matmul(
                                ps[:],
                                xts[dn][:, gt * 128:(gt + 1) * 128],
                                wv_sb[dn][:, hf * 512:(hf + 1) * 512],
                                start=(dn == 0), stop=(dn == NDIN - 1))
                        nc.vector.tensor_tensor(
                            vt[:, hf * 512:(hf + 1) * 512], ps[:],
                            bvr_sb[:, hf * 512:(hf + 1) * 512], ADD)
                    for p in range(NPAIR):
                        nc.tensor.matmul(
                            kvps[p // 2][:, (p % 2) * 256:(p % 2) * 256 + 256],
                            kt[:, p * 128:(p + 1) * 128],
                            vt[:, (p // 2) * 256:(p // 2) * 256 + 256],
                            start=(t == 0 and p % 2 == 0),
                            stop=(t == NTB - 1 and p % 2 == 1))
                # ship only the diagonal [64,64] blocks (head h = 2p+j)
                for p in range(NPAIR):
                    for j in range(2):
                        ex = kvexp.tile([64, 64], CCDT, tag="kvex",
                                        name=f"kvex{b}_{p}_{j}")
                        nc.vector.tensor_copy(
                            ex[:],
                            kvps[p // 2][j * 64:(j + 1) * 64,
                                         (p % 2) * 384 + j * 64:
                                         (p % 2) * 384 + j * 64 + 64])
                        h = 2 * p + j
                        nc.sync.dma_start(
                            bnc_in[b][h * 64:(h + 1) * 64, :], ex[:])
                nc.gpsimd.collective_compute(
                    "AllReduce", mybir.AluOpType.add,
                    replica_groups=[list(range(NCORES))],
                    ins=[bnc_in[b].opt()], outs=[bnc_out[b].opt()])

            # ---- Stage 2/3: Q^T proj, kv readout, fused o-proj ----
            kvsb = {}
            qts = {}

            def prefetch(b):
                for p in range(NPAIR):
                    kv = kvsbp.tile([128, 128], BF, tag="kvsb",
                                    name=f"kvsb{b}_{p}")
                    nc.vector.memset(kv[:], 0.0)
                    for j in range(2):
                        h = 2 * p + j
                        nc.sync.dma_start(
                            kv[j * 64:(j + 1) * 64, j * 64:(j + 1) * 64],
                            bnc_out[b][h * 64:(h + 1) * 64, :])
                    kvsb[(b, p)] = kv

            def qproj(b):
                for p in range(NPAIR):
                    ps = psp.tile([128, 512], f32, tag="ps")
                    for dn in range(NDIN):
                        nc.tensor.matmul(
                            ps[:],
                            wq_sb[dn][:, p * 128:(p + 1) * 128],
                            xts[dn][:, b * 512:(b + 1) * 512],
                            start=(dn == 0), stop=(dn == NDIN - 1))
                    qt = qtp.tile([128, 512], BF, tag="qt",
                                  name=f"qt{b}_{p}")
                    nc.scalar.activation(qt[:], ps[:], RELU,
                                         bias=bq_sb[:, p:p + 1])
                    qts[(b, p)] = qt

            def ro_oproj(b):
                otb = [otbp.tile([128, 512], BF, tag="otb",
                                 name=f"otb{b}_{p}") for p in range(NPAIR)]
                for p in range(NPAIR):
                    pso = psp.tile([128, 512], f32, tag="ps")
                    nc.tensor.matmul(pso[:], kvsb[(b, p)][:], qts[(b, p)][:],
                                     start=True, stop=True)
                    nc.vector.tensor_copy(otb[p][:], pso[:])
                for t in range(NTB):
                    gt = b * NTB + t
                    for hf in range(2):
                        ps = psp.tile([128, 512], f32, tag="ps")
                        for dn in range(NDIN):
                            nc.tensor.matmul(
                                ps[:],
                                otb[dn][:, t * 128:(t + 1) * 128],
                                wo_sb[dn][:, hf * 512:(hf + 1) * 512],
                                start=(dn == 0), stop=(dn == NDIN - 1))
                        yt = ytp.tile([128, 512], BF, tag="yt")
                        nc.vector.tensor_tensor(
                            yt[:], ps[:],
                            bor_sb[:, hf * 512:(hf + 1) * 512], ADD)
                        nc.sync.dma_start(
                            y_d[gt * 128:(gt + 1) * 128,
                                hf * 512:(hf + 1) * 512], yt[:])

            prefetch(0)
            prefetch(1)
            qproj(0)
            qproj(1)
            ro_oproj(0)
            prefetch(2)
            qproj(2)
            ro_oproj(1)
            prefetch(3)
            qproj(3)
            ro_oproj(2)
            ro_oproj(3)

    nc.compile()
    _CACHE["nc_v4"] = nc
    return nc


def prepare_in_maps_v4(x, q_w, q_b, k_w, k_b, v_w, v_b, o_w, o_b):
    import ml_dtypes
    BF = ml_dtypes.bfloat16
    shared = {
        "wq": np.ascontiguousarray(q_w.T).astype(BF),
        "wk": np.ascontiguousarray(k_w.T).astype(BF),
        "wv": np.ascontiguousarray(v_w.T).astype(BF),
        "wo": np.ascontiguousarray(o_w.T).astype(BF),
        "bq": np.ascontiguousarray(q_b.reshape(NDIN, 128).T.astype(np.float32)),
        "bkr": np.ascontiguousarray(
            np.broadcast_to(k_b.reshape(1, D), (128, D))).astype(BF),
        "bvr": np.ascontiguousarray(
            np.broadcast_to(v_b.reshape(1, D), (128, D))).astype(BF),
        "bor": np.ascontiguousarray(
            np.broadcast_to(o_b.reshape(1, D), (128, D))).astype(BF),
    }
    in_maps = []
    for c in range(NCORES):
        xs = x[:, c * SC:(c + 1) * SC, :].reshape(T, D)
        m = dict(shared)
        m["xt"] = np.ascontiguousarray(xs.T).astype(BF)
        in_maps.append(m)
    return in_maps


def gather_output(results):
    y = np.empty((B, S, D), dtype=np.float32)
    for c in range(NCORES):
        y[:, c * SC:(c + 1) * SC, :] = results[c]["y"].astype(
            np.float32).reshape(B, SC, D)
    return y


DTYPE = "v4"


def build_for(dtype):
    return build_program_v4()


def prepare_for(inputs, dtype):
    return prepare_in_maps_v4(**inputs)


def run(inputs, trace=False, dtype=None, **kw):
    from concourse import bass_utils
    dtype = dtype or DTYPE
    nc = build_for(dtype)
    in_maps = prepare_for(inputs, dtype)
    res = bass_utils.run_bass_kernel_spmd(
        nc, in_maps, core_ids=list(range(NCORES)), trace=trace, **kw)
    return gather_output(res.results), res


def kernel(**inputs):
    y, _ = run(inputs)
    return y


# revision 9
# speedup vs baseline: 1.2383x; 1.0042x over previous
"""Trainium2 Bass kernel for MinimalLinearAttention.

  q = relu(x @ q_w.T + q_b); k = relu(x @ k_w.T + k_b); v = x @ v_w.T + v_b
  kv[b,h] = sum_s k[b,s,h,:] outer v[b,s,h,:]          (per batch, all tokens)
  out[b,s,h] = q[b,s,h,:] @ kv[b,h]
  y = out @ o_w.T + o_b

Sharding: token-parallel over 8 cores. Each core takes a 512-token slice of
every batch (2048 tokens), computes k/v projections + partial kv, AllReduces
kv across cores (per batch), then does the q readout + output projection for
its own tokens. Host concatenates slices.

v4 (all-bf16): every matmul operand bf16 (host-cast) so LDWEIGHTS (~105ns)
hides under the ~263ns/512-col matmul issue period. All weights resident in
SBUF. Startup DMAs are spread across the sync/scalar/vector/gpsimd queues
(descriptor issue is ~0.6us per DMA per queue, so one queue serializes the
start) and ordered by first use. No bias matmuls at all: K and V biases ride
the PSUM eviction as DVE tensor_tensor adds against host-replicated bias
tiles (K's relu then happens on the scalar engine from the SBUF temp), O bias
likewise, Q bias is a per-partition activation scalar. y is stored bf16 in
[128,512] halves right after each eviction to shorten the output tail.

Stage 2 is ordered to hide the last kv AllReduce (~25us end-to-end, and the
bounce-buffer reads wait on ALL collectives): two batches of Q-projections
(~34us of kv-independent PE work) run before the first readout; readouts for
a batch run only after all its Q tiles evicted so the PE never stalls on the
qt activation latency.

On-device layouts (per core):
  xt   = x_slice.T            [D=1024, T=2048]   (T cols batch-major: b*512+s)
  wq/wk/wv/wo = W.T           [Din=1024, Dout=1024]
  K, V                        [T, D]     (from xt-stationary matmuls)
  Q^T                         [D, T]     (from w-stationary matmuls)
  kv per (batch, head-pair)   [128, 128] block-diagonal (2 heads of 64)
  y                           [T, D] bf16 (host widens to f32)
"""

import os
import sys

os.environ.setdefault("MYCRO_LOCAL_CACHE", "1")

for _p in ("/opt/trn_rl_repo", "/root/.axon_site/_ro/trn_rl_repo"):
    if os.path.isdir(_p) and _p not in sys.path:
        sys.path.insert(0, _p)

import numpy as np

B, S, D, H, HD = 4, 4096, 1024, 16, 64
NCORES = 8
SC = S // NCORES          # 512 tokens per core per batch
T = B * SC                # 2048 rows per core
NPAIR = 8                 # head pairs (2 heads of 64 dims = 128 partitions)
NDIN = D // 128           # 8 Din tiles
NT = T // 128             # 16 T tiles per core
NTB = SC // 128           # 4 T tiles per batch

CC_BF16 = True            # bf16 kv collective payload

_CACHE = {}


def build_program_v4():
    if "nc_v4" in _CACHE:
        return _CACHE["nc_v4"]

    import concourse.bacc as bacc
    import concourse.tile as tile
    from concourse import bass, mybir

    f32 = mybir.dt.float32
    BF = mybir.dt.bfloat16
    CCDT = BF if CC_BF16 else f32
    RELU = mybir.ActivationFunctionType.Relu
    ADD = mybir.AluOpType.add

    nc = bacc.Bacc("TRN2", target_bir_lowering=False, debug=False,
                   num_devices=NCORES)

    xt_d = nc.dram_tensor("xt", [4, D, 512], BF, kind="ExternalInput").ap()
    wq_d = nc.dram_tensor("wq", [D, D], BF, kind="ExternalInput").ap()
    wk_d = nc.dram_tensor("wk", [D, D], BF, kind="ExternalInput").ap()
    wv_d = nc.dram_tensor("wv", [D, D], BF, kind="ExternalInput").ap()
    wo_d = nc.dram_tensor("wo", [D, D], BF, kind="ExternalInput").ap()
    bq_d = nc.dram_tensor("bq", [128, NDIN], f32, kind="ExternalInput").ap()
    bkr_d = nc.dram_tensor("bkr", [128, D], BF, kind="ExternalInput").ap()
    bvr_d = nc.dram_tensor("bvr", [128, D], BF, kind="ExternalInput").ap()
    bor_d = nc.dram_tensor("bor", [128, D], BF, kind="ExternalInput").ap()
    y_d = nc.dram_tensor("y", [NT, 2, 128, 512], BF,
                         kind="ExternalOutput").ap()

    HPB = 16 * 64  # bounce rows per batch: 16 heads x 64 d-rows

    with tile.TileContext(nc) as tc:
        with (
            tc.tile_pool(name="const", bufs=1) as constp,
            tc.tile_pool(name="wp", bufs=1) as wp,
            tc.tile_pool(name="xtp", bufs=1) as xtp,
            tc.tile_pool(name="kvb", bufs=4) as kvbp,
            tc.tile_pool(name="ktmp", bufs=3) as ktmpp,
            tc.tile_pool(name="qt", bufs=16) as qtp,
            tc.tile_pool(name="otb", bufs=10) as otbp,
            tc.tile_pool(name="kvex", bufs=8) as kvexp,
            tc.tile_pool(name="kvsb", bufs=16) as kvsbp,
            tc.tile_pool(name="yt", bufs=4) as ytp,
            tc.tile_pool(name="dram", bufs=1, space="DRAM") as dramp,
            tc.tile_pool(name="ps", bufs=4, space="PSUM") as psp,
            tc.tile_pool(name="pskv", bufs=4, space="PSUM") as pskvp,
        ):
            # ---- loads: ordered by first use, issue on the 2 HWDGE queues,
            # every bulk DMA reads a contiguous DRAM block ----
            qs = [nc.sync, nc.scalar]
            qi = [0]

            def dma(dst, src):
                qs[qi[0] % 2].dma_start(dst, src)
                qi[0] += 1

            wk_sb = [wp.tile([128, D], BF, tag=f"wk{dn}", name=f"wk_sb{dn}")
                     for dn in range(NDIN)]
            for dn in range(NDIN):
                dma(wk_sb[dn][:], wk_d[dn * 128:(dn + 1) * 128, :])
            xts = [xtp.tile([128, T], BF, tag=f"xt{dn}", name=f"xt_sb{dn}")
                   for dn in range(NDIN)]

            def loadx(q):
                for dn in range(NDIN):
                    dma(xts[dn][:, q * 512:(q + 1) * 512],
                        xt_d[q, dn * 128:(dn + 1) * 128, :])

            loadx(0)

            def loadw(dram_ap, tag):
                w = []
                for dn in range(NDIN):
                    t = wp.tile([128, D], BF, tag=f"{tag}{dn}",
                                name=f"{tag}_sb{dn}")
                    dma(t[:], dram_ap[dn * 128:(dn + 1) * 128, :])
                    w.append(t)
                return w

            wv_sb = loadw(wv_d, "wv")
            bkr_sb = constp.tile([128, D], BF, tag="bkr")
            dma(bkr_sb[:], bkr_d[:])
            bvr_sb = constp.tile([128, D], BF, tag="bvr")
            dma(bvr_sb[:], bvr_d[:])
            loadx(1)
            wq_sb = loadw(wq_d, "wq")
            bq_sb = constp.tile([128, NDIN], f32, tag="bq")
            dma(bq_sb[:], bq_d[:])
            loadx(2)
            loadx(3)
            wo_sb = loadw(wo_d, "wo")
            bor_sb = constp.tile([128, D], BF, tag="bor")
            dma(bor_sb[:], bor_d[:])

            bnc_in = [dramp.tile([HPB, 64], CCDT, tag=f"bi{b}",
                                 name=f"bnc_in{b}") for b in range(B)]
            bnc_out = [dramp.tile([HPB, 64], CCDT, tag=f"bo{b}",
                                  addr_space="Shared", name=f"bnc_out{b}")
                       for b in range(B)]

            # ---- Stage 1: K,V projections + per-batch partial kv ----
            for b in range(B):
                kvps = [pskvp.tile([128, 512], f32, tag="kvps",
                                   name=f"kvps{b}_{w}") for w in range(4)]
                for t in range(NTB):
                    gt = b * NTB + t
                    kt = kvbp.tile([128, D], BF, tag="kb")
                    vt = kvbp.tile([128, D], BF, tag="vb")
                    for hf in range(2):
                        ps = psp.tile([128, 512], f32, tag="ps")
                        for dn in range(NDIN):
                            nc.tensor.matmul(
                                ps[:],
                                xts[dn][:, gt * 128:(gt + 1) * 128],
                                wk_sb[dn][:, hf * 512:(hf + 1) * 512],
                                start=(dn == 0), stop=(dn == NDIN - 1))
                        ktmp = ktmpp.tile([128, 512], BF, tag="ktmp")
                        nc.vector.tensor_tensor(
                            ktmp[:], ps[:],
                            bkr_sb[:, hf * 512:(hf + 1) * 512], ADD)
                        nc.scalar.activation(
                            kt[:, hf * 512:(hf + 1) * 512], ktmp[:], RELU)
                    for hf in range(2):
                        ps = psp.tile([128, 512], f32, tag="ps")
                        for dn in range(NDIN):
                            nc.tensor.matmul(
                                ps[:],
                                xts[dn][:, gt * 128:(gt + 1) * 128],
                                wv_sb[dn][:, hf * 512:(hf + 1) * 512],
                                start=(dn == 0), stop=(dn == NDIN - 1))
                        nc.vector.tensor_tensor(
                            vt[:, hf * 512:(hf + 1) * 512], ps[:],
                            bvr_sb[:, hf * 512:(hf + 1) * 512], ADD)
                    for p in range(NPAIR):
                        nc.tensor.matmul(
                            kvps[p // 2][:, (p % 2) * 256:(p % 2) * 256 + 256],
                            kt[:, p * 128:(p + 1) * 128],
                            vt[:, (p // 2) * 256:(p // 2) * 256 + 256],
                            start=(t == 0 and p % 2 == 0),
                            stop=(t == NTB - 1 and p % 2 == 1))
                # ship only the diagonal [64,64] blocks (head h = 2p+j)
                for p in range(NPAIR):
                    for j in range(2):
                        ex = kvexp.tile([64, 64], CCDT, tag="kvex",
                                        name=f"kvex{b}_{p}_{j}")
                        nc.vector.tensor_copy(
                            ex[:],
                            kvps[p // 2][j * 64:(j + 1) * 64,
                                         (p % 2) * 384 + j * 64:
                                         (p % 2) * 384 + j * 64 + 64])
                        h = 2 * p + j
                        nc.sync.dma_start(
                            bnc_in[b][h * 64:(h + 1) * 64, :], ex[:])
                nc.gpsimd.collective_compute(
                    "AllReduce", mybir.AluOpType.add,
                    replica_groups=[list(range(NCORES))],
                    ins=[bnc_in[b].opt()], outs=[bnc_out[b].opt()])

            # ---- Stage 2/3: Q^T proj, kv readout, fused o-proj ----
            kvsb = {}
            qts = {}

            def prefetch(b):
                for p in range(NPAIR):
                    kv = kvsbp.tile([128, 128], BF, tag="kvsb",
                                    name=f"kvsb{b}_{p}")
                    nc.vector.memset(kv[:], 0.0)
                    for j in range(2):
                        h = 2 * p + j
                        nc.sync.dma_start(
                            kv[j * 64:(j + 1) * 64, j * 64:(j + 1) * 64],
                            bnc_out[b][h * 64:(h + 1) * 64, :])
                    kvsb[(b, p)] = kv

            def qproj(b):
                for p in range(NPAIR):
                    ps = psp.tile([128, 512], f32, tag="ps")
                    for dn in range(NDIN):
                        nc.tensor.matmul(
                            ps[:],
                            wq_sb[dn][:, p * 128:(p + 1) * 128],
                            xts[dn][:, b * 512:(b + 1) * 512],
                            start=(dn == 0), stop=(dn == NDIN - 1))
                    qt = qtp.tile([128, 512], BF, tag="qt",
                                  name=f"qt{b}_{p}")
                    nc.scalar.activation(qt[:], ps[:], RELU,
                                         bias=bq_sb[:, p:p + 1])
                    qts[(b, p)] = qt

            def ro_oproj(b):
                otb = [otbp.tile([128, 512], BF, tag="otb",
                                 name=f"otb{b}_{p}") for p in range(NPAIR)]
                for p in range(NPAIR):
                    pso = psp.tile([128, 512], f32, tag="ps")
                    nc.tensor.matmul(pso[:], kvsb[(b, p)][:], qts[(b, p)][:],
                                     start=True, stop=True)
                    nc.vector.tensor_copy(otb[p][:], pso[:])
                for t in range(NTB):
                    gt = b * NTB + t
                    for hf in range(2):
                        ps = psp.tile([128, 512], f32, tag="ps")
                        for dn in range(NDIN):
                            nc.tensor.matmul(
                                ps[:],
                                otb[dn][:, t * 128:(t + 1) * 128],
                                wo_sb[dn][:, hf * 512:(hf + 1) * 512],
                                start=(dn == 0), stop=(dn == NDIN - 1))
                        yt = ytp.tile([128, 512], BF, tag="yt")
                        nc.vector.tensor_tensor(
                            yt[:], ps[:],
                            bor_sb[:, hf * 512:(hf + 1) * 512], ADD)
                        nc.sync.dma_start(y_d[gt, hf], yt[:])

            prefetch(0)
            prefetch(1)
            qproj(0)
            qproj(1)
            ro_oproj(0)
            prefetch(2)
            qproj(2)
            ro_oproj(1)
            prefetch(3)
            qproj(3)
            ro_oproj(2)
            ro_oproj(3)

    nc.compile()
    _CACHE["nc_v4"] = nc
    return nc


def prepare_in_maps_v4(x, q_w, q_b, k_w, k_b, v_w, v_b, o_w, o_b):
    import ml_dtypes
    BF = ml_dtypes.bfloat16
    shared = {
        "wq": np.ascontiguousarray(q_w.T).astype(BF),
        "wk": np.ascontiguousarray(k_w.T).astype(BF),
        "wv": np.ascontiguousarray(v_w.T).astype(BF),
        "wo": np.ascontiguousarray(o_w.T).astype(BF),
        "bq": np.ascontiguousarray(q_b.reshape(NDIN, 128).T.astype(np.float32)),
        "bkr": np.ascontiguousarray(
            np.broadcast_to(k_b.reshape(1, D), (128, D))).astype(BF),
        "bvr": np.ascontiguousarray(
            np.broadcast_to(v_b.reshape(1, D), (128, D))).astype(BF),
        "bor": np.ascontiguousarray(
            np.broadcast_to(o_b.reshape(1, D), (128, D))).astype(BF),
    }
    in_maps = []
    for c in range(NCORES):
        xs = x[:, c * SC:(c + 1) * SC, :].reshape(T, D)
        xt = xs.T  # [D, T]
        m = dict(shared)
        # quarter-major: [4, D, 512] so each [128, 512] chunk is contiguous
        m["xt"] = np.ascontiguousarray(
            xt.reshape(D, 4, 512).transpose(1, 0, 2)).astype(BF)
        in_maps.append(m)
    return in_maps


def gather_output(results):
    y = np.empty((B, S, D), dtype=np.float32)
    for c in range(NCORES):
        # y_d is [NT, 2, 128, 512] -> [NT*128, 1024]
        yc = results[c]["y"].astype(np.float32)
        yc = yc.transpose(0, 2, 1, 3).reshape(T, D)
        y[:, c * SC:(c + 1) * SC, :] = yc.reshape(B, SC, D)
    return y


DTYPE = "v4"


def build_for(dtype):
    return build_program_v4()


def prepare_for(inputs, dtype):
    return prepare_in_maps_v4(**inputs)


def run(inputs, trace=False, dtype=None, **kw):
    from concourse import bass_utils
    dtype = dtype or DTYPE
    nc = build_for(dtype)
    in_maps = prepare_for(inputs, dtype)
    res = bass_utils.run_bass_kernel_spmd(
        nc, in_maps, core_ids=list(range(NCORES)), trace=trace, **kw)
    return gather_output(res.results), res


def kernel(**inputs):
    y, _ = run(inputs)
    return y


# revision 10
# speedup vs baseline: 1.2713x; 1.0266x over previous
"""Trainium2 Bass kernel for MinimalLinearAttention.

  q = relu(x @ q_w.T + q_b); k = relu(x @ k_w.T + k_b); v = x @ v_w.T + v_b
  kv[b,h] = sum_s k[b,s,h,:] outer v[b,s,h,:]          (per batch, all tokens)
  out[b,s,h] = q[b,s,h,:] @ kv[b,h]
  y = out @ o_w.T + o_b

Sharding: token-parallel over 8 cores. Each core takes a 512-token slice of
every batch (2048 tokens), computes k/v projections + partial kv, AllReduces
kv across cores (per batch), then does the q readout + output projection for
its own tokens. Host concatenates slices.

v6 (all-bf16). Matmul operands bf16 so LDWEIGHTS (~105ns) hides under the
~263ns/512-col matmul issue period; all weights resident in SBUF. Input DMAs
are the startup bottleneck (descriptor issue is ~0.7us per DMA on a queue),
so they are minimized and ordered by first use on the two HWDGE queues:
wk (8), x tokens 0:1024 (8, contiguous halves), wv (8), all replicated
biases (1), x tokens 1024:2048 (8), wq|wo merged per-din (8), bq (1).
No bias matmuls: K/V/O biases ride the PSUM eviction as DVE tensor_tensor
adds against host-replicated rows (K's relu then runs on the scalar engine);
Q bias is a per-partition activation scalar. The otb eviction runs on the
scalar engine to keep Vector off the stage-3 critical path. y is stored bf16
in [128,512] halves, alternating sync/scalar queues, DRAM-contiguous.

Stage 2 is ordered to hide the last kv AllReduce (~25us end-to-end, and the
bounce-buffer reads wait on ALL collectives): two batches of Q-projections
(~34us of kv-independent PE work) run before the first readout; readouts for
a batch run only after all its Q tiles evicted so the PE never stalls on the
qt activation latency.

On-device layouts (per core):
  xt   = x_slice.T            [2, D, 1024] halves (cols batch-major b*512+s)
  w*   = W.T                  [Din=1024, Dout=1024] (wq|wo merged [D, 2048])
  K, V                        [T, D]     (from xt-stationary matmuls)
  Q^T                         [D, T]     (from w-stationary matmuls)
  kv per (batch, head-pair)   [128, 128] block-diagonal (2 heads of 64)
  y                           [NT, 2, 128, 512] bf16 (host reassembles f32)
"""

import os
import sys

os.environ.setdefault("MYCRO_LOCAL_CACHE", "1")

for _p in ("/opt/trn_rl_repo", "/root/.axon_site/_ro/trn_rl_repo"):
    if os.path.isdir(_p) and _p not in sys.path:
        sys.path.insert(0, _p)

import numpy as np

B, S, D, H, HD = 4, 4096, 1024, 16, 64
NCORES = 8
SC = S // NCORES          # 512 tokens per core per batch
T = B * SC                # 2048 rows per core
NPAIR = 8                 # head pairs (2 heads of 64 dims = 128 partitions)
NDIN = D // 128           # 8 Din tiles
NT = T // 128             # 16 T tiles per core
NTB = SC // 128           # 4 T tiles per batch

CC_BF16 = True            # bf16 kv collective payload

_CACHE = {}


def build_program_v6():
    if "nc_v6" in _CACHE:
        return _CACHE["nc_v6"]

    import concourse.bacc as bacc
    import concourse.tile as tile
    from concourse import bass, mybir

    f32 = mybir.dt.float32
    BF = mybir.dt.bfloat16
    CCDT = BF if CC_BF16 else f32
    RELU = mybir.ActivationFunctionType.Relu
    COPY = mybir.ActivationFunctionType.Copy
    ADD = mybir.AluOpType.add

    nc = bacc.Bacc("TRN2", target_bir_lowering=False, debug=False,
                   num_devices=NCORES)

    xt_d = nc.dram_tensor("xt", [2, D, 1024], BF, kind="ExternalInput").ap()
    wk_d = nc.dram_tensor("wk", [D, D], BF, kind="ExternalInput").ap()
    wv_d = nc.dram_tensor("wv", [D, D], BF, kind="ExternalInput").ap()
    wqo_d = nc.dram_tensor("wqo", [D, 2 * D], BF, kind="ExternalInput").ap()
    bq_d = nc.dram_tensor("bq", [128, NDIN], f32, kind="ExternalInput").ap()
    # bkvo = [bkr | bvr | bor], each [128, D] replicated rows
    bkvo_d = nc.dram_tensor("bkvo", [128, 3 * D], BF,
                            kind="ExternalInput").ap()
    y_d = nc.dram_tensor("y", [NT, 2, 128, 512], BF,
                         kind="ExternalOutput").ap()

    HPB = 16 * 64  # bounce rows per batch: 16 heads x 64 d-rows

    with tile.TileContext(nc) as tc:
        with (
            tc.tile_pool(name="const", bufs=1) as constp,
            tc.tile_pool(name="wp", bufs=1) as wp,
            tc.tile_pool(name="xtp", bufs=1) as xtp,
            tc.tile_pool(name="kvb", bufs=4) as kvbp,
            tc.tile_pool(name="ktmp", bufs=3) as ktmpp,
            tc.tile_pool(name="qt", bufs=16) as qtp,
            tc.tile_pool(name="otb", bufs=10) as otbp,
            tc.tile_pool(name="kvex", bufs=8) as kvexp,
            tc.tile_pool(name="kvsb", bufs=16) as kvsbp,
            tc.tile_pool(name="yt", bufs=4) as ytp,
            tc.tile_pool(name="dram", bufs=1, space="DRAM") as dramp,
            tc.tile_pool(name="ps", bufs=4, space="PSUM") as psp,
            tc.tile_pool(name="pskv", bufs=4, space="PSUM") as pskvp,
        ):
            # ---- loads: first-use order, alternating the 2 HWDGE queues ----
            qs = [nc.sync, nc.scalar]
            qi = [0]

            def dma(dst, src):
                qs[qi[0] % 2].dma_start(dst, src)
                qi[0] += 1

            wk_sb = [wp.tile([128, D], BF, tag=f"wk{dn}", name=f"wk_sb{dn}")
                     for dn in range(NDIN)]
            for dn in range(NDIN):
                dma(wk_sb[dn][:], wk_d[dn * 128:(dn + 1) * 128, :])
            xts = [xtp.tile([128, T], BF, tag=f"xt{dn}", name=f"xt_sb{dn}")
                   for dn in range(NDIN)]

            def loadx(hx):
                for dn in range(NDIN):
                    dma(xts[dn][:, hx * 1024:(hx + 1) * 1024],
                        xt_d[hx, dn * 128:(dn + 1) * 128, :])

            loadx(0)
            wv_sb = [wp.tile([128, D], BF, tag=f"wv{dn}", name=f"wv_sb{dn}")
                     for dn in range(NDIN)]
            for dn in range(NDIN):
                dma(wv_sb[dn][:], wv_d[dn * 128:(dn + 1) * 128, :])
            bkvo_sb = constp.tile([128, 3 * D], BF, tag="bkvo")
            dma(bkvo_sb[:], bkvo_d[:])
            bkr_sb = bkvo_sb[:, 0:D]
            bvr_sb = bkvo_sb[:, D:2 * D]
            bor_sb = bkvo_sb[:, 2 * D:3 * D]
            loadx(1)
            wqo_sb = [wp.tile([128, 2 * D], BF, tag=f"wqo{dn}",
                              name=f"wqo_sb{dn}") for dn in range(NDIN)]
            for dn in range(NDIN):
                dma(wqo_sb[dn][:], wqo_d[dn * 128:(dn + 1) * 128, :])
            wq_sb = [wqo_sb[dn][:, 0:D] for dn in range(NDIN)]
            wo_sb = [wqo_sb[dn][:, D:2 * D] for dn in range(NDIN)]
            bq_sb = constp.tile([128, NDIN], f32, tag="bq")
            dma(bq_sb[:], bq_d[:])

            bnc_in = [dramp.tile([HPB, 64], CCDT, tag=f"bi{b}",
                                 name=f"bnc_in{b}") for b in range(B)]
            bnc_out = [dramp.tile([HPB, 64], CCDT, tag=f"bo{b}",
                                  addr_space="Shared", name=f"bnc_out{b}")
                       for b in range(B)]

            # ---- Stage 1: K,V projections + per-batch partial kv ----
            for b in range(B):
                kvps = [pskvp.tile([128, 512], f32, tag="kvps",
                                   name=f"kvps{b}_{w}") for w in range(4)]
                for t in range(NTB):
                    gt = b * NTB + t
                    kt = kvbp.tile([128, D], BF, tag="kb")
                    vt = kvbp.tile([128, D], BF, tag="vb")
                    for hf in range(2):
                        ps = psp.tile([128, 512], f32, tag="ps")
                        for dn in range(NDIN):
                            nc.tensor.matmul(
                                ps[:],
                                xts[dn][:, gt * 128:(gt + 1) * 128],
                                wk_sb[dn][:, hf * 512:(hf + 1) * 512],
                                start=(dn == 0), stop=(dn == NDIN - 1))
                        ktmp = ktmpp.tile([128, 512], BF, tag="ktmp")
                        nc.vector.tensor_tensor(
                            ktmp[:], ps[:],
                            bkr_sb[:, hf * 512:(hf + 1) * 512], ADD)
                        nc.scalar.activation(
                            kt[:, hf * 512:(hf + 1) * 512], ktmp[:], RELU)
                    for hf in range(2):
                        ps = psp.tile([128, 512], f32, tag="ps")
                        for dn in range(NDIN):
                            nc.tensor.matmul(
                                ps[:],
                                xts[dn][:, gt * 128:(gt + 1) * 128],
                                wv_sb[dn][:, hf * 512:(hf + 1) * 512],
                                start=(dn == 0), stop=(dn == NDIN - 1))
                        nc.vector.tensor_tensor(
                            vt[:, hf * 512:(hf + 1) * 512], ps[:],
                            bvr_sb[:, hf * 512:(hf + 1) * 512], ADD)
                    for p in range(NPAIR):
                        nc.tensor.matmul(
                            kvps[p // 2][:, (p % 2) * 256:(p % 2) * 256 + 256],
                            kt[:, p * 128:(p + 1) * 128],
                            vt[:, (p // 2) * 256:(p // 2) * 256 + 256],
                            start=(t == 0 and p % 2 == 0),
                            stop=(t == NTB - 1 and p % 2 == 1))
                # ship only the diagonal [64,64] blocks (head h = 2p+j)
                for p in range(NPAIR):
                    for j in range(2):
                        ex = kvexp.tile([64, 64], CCDT, tag="kvex",
                                        name=f"kvex{b}_{p}_{j}")
                        nc.vector.tensor_copy(
                            ex[:],
                            kvps[p // 2][j * 64:(j + 1) * 64,
                                         (p % 2) * 384 + j * 64:
                                         (p % 2) * 384 + j * 64 + 64])
                        h = 2 * p + j
                        nc.sync.dma_start(
                            bnc_in[b][h * 64:(h + 1) * 64, :], ex[:])
                nc.gpsimd.collective_compute(
                    "AllReduce", mybir.AluOpType.add,
                    replica_groups=[list(range(NCORES))],
                    ins=[bnc_in[b].opt()], outs=[bnc_out[b].opt()])

            # ---- Stage 2/3: Q^T proj, kv readout, fused o-proj ----
            kvsb = {}
            qts = {}

            def prefetch(b):
                for p in range(NPAIR):
                    kv = kvsbp.tile([128, 128], BF, tag="kvsb",
                                    name=f"kvsb{b}_{p}")
                    nc.vector.memset(kv[:], 0.0)
                    for j in range(2):
                        h = 2 * p + j
                        nc.sync.dma_start(
                            kv[j * 64:(j + 1) * 64, j * 64:(j + 1) * 64],
                            bnc_out[b][h * 64:(h + 1) * 64, :])
                    kvsb[(b, p)] = kv

            def qproj(b):
                for p in range(NPAIR):
                    ps = psp.tile([128, 512], f32, tag="ps")
                    for dn in range(NDIN):
                        nc.tensor.matmul(
                            ps[:],
                            wq_sb[dn][:, p * 128:(p + 1) * 128],
                            xts[dn][:, b * 512:(b + 1) * 512],
                            start=(dn == 0), stop=(dn == NDIN - 1))
                    qt = qtp.tile([128, 512], BF, tag="qt",
                                  name=f"qt{b}_{p}")
                    nc.scalar.activation(qt[:], ps[:], RELU,
                                         bias=bq_sb[:, p:p + 1])
                    qts[(b, p)] = qt

            def ro_oproj(b):
                otb = [otbp.tile([128, 512], BF, tag="otb",
                                 name=f"otb{b}_{p}") for p in range(NPAIR)]
                for p in range(NPAIR):
                    pso = psp.tile([128, 512], f32, tag="ps")
                    nc.tensor.matmul(pso[:], kvsb[(b, p)][:], qts[(b, p)][:],
                                     start=True, stop=True)
                    nc.scalar.activation(otb[p][:], pso[:], COPY)
                for t in range(NTB):
                    gt = b * NTB + t
                    for hf in range(2):
                        ps = psp.tile([128, 512], f32, tag="ps")
                        for dn in range(NDIN):
                            nc.tensor.matmul(
                                ps[:],
                                otb[dn][:, t * 128:(t + 1) * 128],
                                wo_sb[dn][:, hf * 512:(hf + 1) * 512],
                                start=(dn == 0), stop=(dn == NDIN - 1))
                        yt = ytp.tile([128, 512], BF, tag="yt")
                        nc.vector.tensor_tensor(
                            yt[:], ps[:],
                            bor_sb[:, hf * 512:(hf + 1) * 512], ADD)
                        qs[(gt * 2 + hf) % 2].dma_start(y_d[gt, hf], yt[:])

            prefetch(0)
            prefetch(1)
            qproj(0)
            qproj(1)
            ro_oproj(0)
            prefetch(2)
            qproj(2)
            ro_oproj(1)
            prefetch(3)
            qproj(3)
            ro_oproj(2)
            ro_oproj(3)

    nc.compile()
    _CACHE["nc_v6"] = nc
    return nc


def prepare_in_maps_v6(x, q_w, q_b, k_w, k_b, v_w, v_b, o_w, o_b):
    import ml_dtypes
    BF = ml_dtypes.bfloat16
    wqo = np.concatenate([q_w.T, o_w.T], axis=1)  # [D, 2D]
    bkvo = np.concatenate([
        np.broadcast_to(k_b.reshape(1, D), (128, D)),
        np.broadcast_to(v_b.reshape(1, D), (128, D)),
        np.broadcast_to(o_b.reshape(1, D), (128, D)),
    ], axis=1)  # [128, 3D]
    shared = {
        "wk": np.ascontiguousarray(k_w.T).astype(BF),
        "wv": np.ascontiguousarray(v_w.T).astype(BF),
        "wqo": np.ascontiguousarray(wqo).astype(BF),
        "bq": np.ascontiguousarray(q_b.reshape(NDIN, 128).T.astype(np.float32)),
        "bkvo": np.ascontiguousarray(bkvo).astype(BF),
    }
    in_maps = []
    for c in range(NCORES):
        xs = x[:, c * SC:(c + 1) * SC, :].reshape(T, D)
        xt = xs.T  # [D, T]
        m = dict(shared)
        # halves: [2, D, 1024] so each [128, 1024] chunk is contiguous
        m["xt"] = np.ascontiguousarray(
            xt.reshape(D, 2, 1024).transpose(1, 0, 2)).astype(BF)
        in_maps.append(m)
    return in_maps


def gather_output(results):
    y = np.empty((B, S, D), dtype=np.float32)
    for c in range(NCORES):
        # y_d is [NT, 2, 128, 512] -> [NT*128, 1024]
        yc = results[c]["y"].astype(np.float32)
        yc = yc.transpose(0, 2, 1, 3).reshape(T, D)
        y[:, c * SC:(c + 1) * SC, :] = yc.reshape(B, SC, D)
    return y


DTYPE = "v6"


def build_for(dtype):
    return build_program_v6()


def prepare_for(inputs, dtype):
    return prepare_in_maps_v6(**inputs)


def run(inputs, trace=False, dtype=None, **kw):
    from concourse import bass_utils
    dtype = dtype or DTYPE
    nc = build_for(dtype)
    in_maps = prepare_for(inputs, dtype)
    res = bass_utils.run_bass_kernel_spmd(
        nc, in_maps, core_ids=list(range(NCORES)), trace=trace, **kw)
    return gather_output(res.results), res


def kernel(**inputs):
    y, _ = run(inputs)
    return y
